# revision 6
# baseline (speedup 1.0000x reference)
"""GQA (32 q heads / 8 kv heads, RoPE, causal) sharded over 8 NeuronCores.

Sharding: tensor-parallel over kv-head groups. Core d owns q heads
4d..4d+3 and kv head d. w_in is sharded by rows (column-parallel in
torch convention), w_out by columns; per-core partial outputs of the
out-projection are summed on the host.

Per-core kernel (all matmuls bf16, fp32 accumulation):
  phase A: qkv = x @ w_in_shard.T  (token-major, x tiles stationary)
           + RoPE on q/k (DVE) + DMA-transpose q/k to head-dim-major
  phase B: scoresT[tk,tq] = k^T q;  probs = exp(scores/sqrt(D)) with
           causal zeroing post-exp (affine_select); p@v accumulates
           [tq, D] plus a ones-column giving the softmax denominator
  phase C: partial_out[o, t] = w_out_shard^T @ attn  (w_out stationary)
"""

import numpy as np
import ml_dtypes

import concourse.bacc as bacc
import concourse.mybir as mybir
import concourse.tile as tile

BF16 = ml_dtypes.bfloat16

N_CORES = 8
B, S, E = 2, 2048, 4096
D = 128            # head dim
HL = 4             # local q heads per core
NT = S // 128      # 16 token blocks per batch
NE = E // 128      # 32 embedding blocks
NS = S // 512      # 4 token superblocks per batch
F = HL * D + 2 * D  # 768 local qkv features
SCALE = float(1.0 / np.sqrt(D))


def build_program():
    nc = bacc.Bacc("TRN2", target_bir_lowering=False, debug=False,
                   num_devices=N_CORES)
    bf = mybir.dt.bfloat16
    f32 = mybir.dt.float32

    xT = nc.dram_tensor("xT", [E, B * S], bf, kind="ExternalInput").ap()
    wq = nc.dram_tensor("wq", [E, F], bf, kind="ExternalInput").ap()
    wo = nc.dram_tensor("wo", [HL * D, E], bf, kind="ExternalInput").ap()
    cosd = nc.dram_tensor("cosd", [S, HL * 64], bf, kind="ExternalInput").ap()
    sind = nc.dram_tensor("sind", [S, HL * 64], bf, kind="ExternalInput").ap()
    po = nc.dram_tensor("po", [E, B * S], f32, kind="ExternalOutput").ap()

    with tile.TileContext(nc) as tc:
        with (
            tc.tile_pool(name="const", bufs=1) as constp,
            tc.tile_pool(name="xcol", bufs=3) as xcolp,
            tc.tile_pool(name="qtm", bufs=3) as qtmp,
            tc.tile_pool(name="ktm", bufs=3) as ktmp,
            tc.tile_pool(name="tmp", bufs=6) as tmpp,
            tc.tile_pool(name="wot", bufs=3) as wotp,
            tc.tile_pool(name="qT", bufs=2) as qTp,
            tc.tile_pool(name="kT", bufs=2) as kTp,
            tc.tile_pool(name="vh", bufs=2) as vhp,
            tc.tile_pool(name="afm", bufs=2) as afmp,
            tc.tile_pool(name="probs", bufs=4) as probsp,
            tc.tile_pool(name="atm", bufs=4) as atmp,
            tc.tile_pool(name="rcp", bufs=4) as rcpp,
            tc.tile_pool(name="osb", bufs=4) as osbp,
            tc.tile_pool(name="ps", bufs=8, space="PSUM") as psp,
        ):
            # resident weights / tables
            wq_sb = constp.tile([128, NE, F], bf)
            nc.sync.dma_start(wq_sb[:], wq.rearrange("(eb p) f -> p eb f", p=128))
            cos_sb = constp.tile([128, NT, HL * 64], bf)
            nc.sync.dma_start(cos_sb[:], cosd.rearrange("(tb p) c -> p tb c", p=128))
            sin_sb = constp.tile([128, NT, HL * 64], bf)
            nc.sync.dma_start(sin_sb[:], sind.rearrange("(tb p) c -> p tb c", p=128))

            for b in range(B):
                # ---------------- phase A: qkv projection + RoPE ----------
                qT = qTp.tile([128, HL, S], bf)      # q head-dim-major
                kT = kTp.tile([128, S], bf)          # k head-dim-major
                vh = vhp.tile([128, NT, 132], bf)    # v token-major + ones col
                nc.gpsimd.memset(vh[:, :, 128:129], 1.0)

                for tb in range(NT):
                    col = b * NT + tb
                    xcol = xcolp.tile([128, NE, 128], bf)
                    nc.sync.dma_start(
                        xcol[:],
                        xT[:, col * 128:(col + 1) * 128].rearrange(
                            "(eb p) t -> p eb t", p=128),
                    )
                    psq = psp.tile([128, 512], mybir.dt.float32, tag="ps")
                    pskv = psp.tile([128, 256], mybir.dt.float32, tag="ps")
                    for eb in range(NE):
                        nc.tensor.matmul(psq[:], xcol[:, eb, :], wq_sb[:, eb, 0:512],
                                         start=(eb == 0), stop=(eb == NE - 1))
                        nc.tensor.matmul(pskv[:], xcol[:, eb, :], wq_sb[:, eb, 512:F],
                                         start=(eb == 0), stop=(eb == NE - 1))

                    # RoPE on q (4 heads batched) and k; v copied straight.
                    psq3 = psq[:].rearrange("p (h z) -> p h z", h=HL)
                    qlo, qhi = psq3[:, :, 0:64], psq3[:, :, 64:128]
                    cos_t = cos_sb[:, tb, :].rearrange("p (h z) -> p h z", h=HL)
                    sin_t = sin_sb[:, tb, :].rearrange("p (h z) -> p h z", h=HL)
                    qtm = qtmp.tile([128, HL, 128], bf)
                    ta = tmpp.tile([128, HL, 64], mybir.dt.float32, tag="rt")
                    tb_ = tmpp.tile([128, HL, 64], mybir.dt.float32, tag="rt")
                    nc.vector.tensor_mul(ta[:], qlo, cos_t)
                    nc.vector.tensor_mul(tb_[:], qhi, sin_t)
                    nc.vector.tensor_sub(qtm[:, :, 0:64], ta[:], tb_[:])
                    tc_ = tmpp.tile([128, HL, 64], mybir.dt.float32, tag="rt")
                    td = tmpp.tile([128, HL, 64], mybir.dt.float32, tag="rt")
                    nc.vector.tensor_mul(tc_[:], qhi, cos_t)
                    nc.vector.tensor_mul(td[:], qlo, sin_t)
                    nc.vector.tensor_add(qtm[:, :, 64:128], tc_[:], td[:])

                    klo, khi = pskv[:, 0:64], pskv[:, 64:128]
                    cos_k = cos_sb[:, tb, 0:64]
                    sin_k = sin_sb[:, tb, 0:64]
                    ktm = ktmp.tile([128, 128], bf)
                    ka = tmpp.tile([128, 64], mybir.dt.float32, tag="rt")
                    kb = tmpp.tile([128, 64], mybir.dt.float32, tag="rt")
                    nc.vector.tensor_mul(ka[:], klo, cos_k)
                    nc.vector.tensor_mul(kb[:], khi, sin_k)
                    nc.vector.tensor_sub(ktm[:, 0:64], ka[:], kb[:])
                    kc = tmpp.tile([128, 64], mybir.dt.float32, tag="rt")
                    kd = tmpp.tile([128, 64], mybir.dt.float32, tag="rt")
                    nc.vector.tensor_mul(kc[:], khi, cos_k)
                    nc.vector.tensor_mul(kd[:], klo, sin_k)
                    nc.vector.tensor_add(ktm[:, 64:128], kc[:], kd[:])

                    nc.any.tensor_copy(vh[:, tb, 0:128], pskv[:, 128:256])

                    # transpose to head-dim-major via DMA transpose
                    for h in range(HL):
                        nc.scalar.dma_start_transpose(
                            qT[:, h, tb * 128:(tb + 1) * 128], qtm[:, h, :])
                    nc.scalar.dma_start_transpose(
                        kT[:, tb * 128:(tb + 1) * 128], ktm[:])

                # ---------------- phase B: attention ----------------------
                afm = afmp.tile([128, HL, S], bf)    # attn out, feature-major
                for h in range(HL):
                    for s in range(NS):
                        pvs = [psp.tile([128, 132], mybir.dt.float32, tag="ps",
                                        name=f"pv{u}")
                               for u in range(4)]
                        for j in range(4 * s + 4):
                            pscore = psp.tile([128, 512], mybir.dt.float32,
                                              tag="ps")
                            nc.tensor.matmul(
                                pscore[:],
                                kT[:, j * 128:(j + 1) * 128],
                                qT[:, h, s * 512:(s + 1) * 512],
                                start=True, stop=True)
                            probs = probsp.tile([128, 512], bf)
                            nc.scalar.activation(
                                probs[:], pscore[:],
                                mybir.ActivationFunctionType.Exp, scale=SCALE)
                            delta = j * 128 - s * 512
                            if delta > -128:
                                # in-tile causal mask: keep col - row >= delta
                                nc.gpsimd.affine_select(
                                    out=probs[:], in_=probs[:],
                                    compare_op=mybir.AluOpType.is_ge,
                                    fill=0.0, base=-delta,
                                    pattern=[[1, 512]], channel_multiplier=-1)
                            for u in range(4):
                                if 4 * s + u < j:
                                    continue
                                nc.tensor.matmul(
                                    pvs[u][:, 0:129],
                                    probs[:, u * 128:(u + 1) * 128],
                                    vh[:, j, 0:129],
                                    start=(j == 0), stop=(j == 4 * s + u))
                        for u in range(4):
                            tq = s * 4 + u
                            rcp = rcpp.tile([128, 1], mybir.dt.float32)
                            nc.vector.reciprocal(rcp[:], pvs[u][:, 128:129])
                            atm = atmp.tile([128, 128], bf)
                            nc.vector.tensor_scalar_mul(
                                atm[:], pvs[u][:, 0:128], rcp[:])
                            nc.scalar.dma_start_transpose(
                                afm[:, h, tq * 128:(tq + 1) * 128], atm[:])

                # ---------------- phase C: out projection -----------------
                for ob in range(E // 128):
                    wot = wotp.tile([128, HL, 128], bf)
                    nc.sync.dma_start(
                        wot[:],
                        wo[:, ob * 128:(ob + 1) * 128].rearrange(
                            "(h p) o -> p h o", p=128))
                    for ts in range(NS):
                        pso = psp.tile([128, 512], mybir.dt.float32, tag="ps")
                        for h in range(HL):
                            nc.tensor.matmul(
                                pso[:],
                                wot[:, h, :],
                                afm[:, h, ts * 512:(ts + 1) * 512],
                                start=(h == 0), stop=(h == HL - 1))
                        osb = osbp.tile([128, 512], mybir.dt.float32)
                        nc.any.tensor_copy(osb[:], pso[:])
                        nc.sync.dma_start(
                            po[ob * 128:(ob + 1) * 128,
                               b * S + ts * 512: b * S + (ts + 1) * 512],
                            osb[:])

    nc.compile()
    return nc


def prep_inputs(x, w_in, w_out):
    """Host-side shard + layout prep. Returns in_maps for the 8 cores."""
    x = np.asarray(x, dtype=np.float32)
    w_in = np.asarray(w_in, dtype=np.float32)
    w_out = np.asarray(w_out, dtype=np.float32)

    xT = np.ascontiguousarray(x.reshape(B * S, E).T).astype(BF16)

    # RoPE tables, fp32 math to match the reference
    inv_freq = (1.0 / (10000.0 ** (np.arange(0, D, 2, dtype=np.float32) / D))
                ).astype(np.float32)
    ang = np.arange(S, dtype=np.float32)[:, None] * inv_freq[None, :]
    cos1 = np.cos(ang).astype(np.float32)   # [S, 64]
    sin1 = np.sin(ang).astype(np.float32)
    cos4 = np.tile(cos1, (1, HL)).astype(BF16)   # [S, 256]
    sin4 = np.tile(sin1, (1, HL)).astype(BF16)

    in_maps = []
    for d in range(N_CORES):
        rows = np.concatenate([
            w_in[512 * d: 512 * d + 512],
            w_in[4096 + 128 * d: 4096 + 128 * d + 128],
            w_in[5120 + 128 * d: 5120 + 128 * d + 128],
        ], axis=0)                                    # [768, E]
        wq_d = np.ascontiguousarray(rows.T).astype(BF16)      # [E, 768]
        wo_d = np.ascontiguousarray(
            w_out[:, 512 * d: 512 * d + 512].T).astype(BF16)  # [512, E]
        in_maps.append({"xT": xT, "wq": wq_d, "wo": wo_d,
                        "cosd": cos4, "sind": sin4})
    return in_maps


_NC_CACHE = None


def get_program():
    global _NC_CACHE
    if _NC_CACHE is None:
        _NC_CACHE = build_program()
    return _NC_CACHE


def kernel(x, w_in, w_out):
    from concourse.bass_utils import run_bass_kernel_spmd
    nc = get_program()
    in_maps = prep_inputs(x, w_in, w_out)
    res = run_bass_kernel_spmd(nc, in_maps, list(range(N_CORES)))
    total = np.zeros((E, B * S), dtype=np.float32)
    for d in range(N_CORES):
        total += res.results[d]["po"]
    out = total.reshape(E, B, S).transpose(1, 2, 0)
    return np.ascontiguousarray(out, dtype=np.float32)


# revision 7
# speedup vs baseline: 9.4046x; 9.4046x over previous
"""GQA (32 q heads / 8 kv heads, RoPE, causal) sharded over 8 NeuronCores.

Sharding: tensor-parallel over kv-head groups. Core d owns q heads
4d..4d+3 and kv head d. w_in is sharded by rows (column-parallel in
torch convention), w_out by columns; per-core partial outputs of the
out-projection are summed on the host.

Per-core kernel (all matmuls bf16, fp32 accumulation):
  phase A: qkv = x @ w_in_shard.T  (token-major, x tiles stationary)
           + RoPE on q/k (DVE) + DMA-transpose q/k to head-dim-major
  phase B: scoresT[tk,tq] = k^T q;  probs = exp(scores/sqrt(D)) with
           causal zeroing post-exp (affine_select); p@v accumulates
           [tq, D] plus a ones-column giving the softmax denominator
  phase C: partial_out[o, t] = w_out_shard^T @ attn  (w_out stationary)
"""

import numpy as np
import ml_dtypes

import concourse.bacc as bacc
import concourse.mybir as mybir
import concourse.tile as tile

BF16 = ml_dtypes.bfloat16

N_CORES = 8
B, S, E = 2, 2048, 4096
D = 128            # head dim
HL = 4             # local q heads per core
NT = S // 128      # 16 token blocks per batch
NE = E // 128      # 32 embedding blocks
NS = S // 512      # 4 token superblocks per batch
F = HL * D + 2 * D  # 768 local qkv features
SCALE = float(1.0 / np.sqrt(D))

bf = mybir.dt.bfloat16
f32 = mybir.dt.float32


def _emit_iter(nc, P, ap):
    """One full forward pass (both batches)."""
    wq_sb = P["const"].tile([128, NE, F], bf, name="wq_sb")
    nc.sync.dma_start(wq_sb[:], ap["wq"].rearrange("(eb p) f -> p eb f", p=128))
    cos_sb = P["const"].tile([128, NT, HL * 64], bf, name="cos_sb")
    nc.sync.dma_start(cos_sb[:], ap["cosd"].rearrange("(tb p) c -> p tb c", p=128))
    sin_sb = P["const"].tile([128, NT, HL * 64], bf, name="sin_sb")
    nc.sync.dma_start(sin_sb[:], ap["sind"].rearrange("(tb p) c -> p tb c", p=128))

    for b in range(B):
        qT, kT, vh = _emit_qkv(nc, P, ap, b, wq_sb, cos_sb, sin_sb)
        afm = _emit_attn(nc, P, qT, kT, vh)
        _emit_outproj(nc, P, ap, b, afm)


def _emit_qkv(nc, P, ap, b, wq_sb, cos_sb, sin_sb):
    qT = P["qT"].tile([128, HL, S], bf, name="qT")      # q head-dim-major
    kT = P["kT"].tile([128, S], bf, name="kT")          # k head-dim-major
    vh = P["vh"].tile([128, NT, 132], bf, name="vh")    # v token-major + ones
    nc.gpsimd.memset(vh[:, :, 128:129], 1.0)

    for tb in range(NT):
        col = b * NT + tb
        xcol = P["xcol"].tile([128, NE, 128], bf, name="xcol")
        nc.sync.dma_start(
            xcol[:],
            ap["xT"][:, col * 128:(col + 1) * 128].rearrange(
                "(eb p) t -> p eb t", p=128))
        psq = P["ps"].tile([128, 512], f32, tag="ps", name="psq")
        pskv = P["ps"].tile([128, 256], f32, tag="ps", name="pskv")
        for eb in range(NE):
            nc.tensor.matmul(psq[:], xcol[:, eb, :], wq_sb[:, eb, 0:512],
                             start=(eb == 0), stop=(eb == NE - 1))
            nc.tensor.matmul(pskv[:], xcol[:, eb, :], wq_sb[:, eb, 512:F],
                             start=(eb == 0), stop=(eb == NE - 1))

        # RoPE on q (4 heads batched via strided APs) and on k.
        psq3 = psq[:].rearrange("p (h z) -> p h z", h=HL)
        qlo, qhi = psq3[:, :, 0:64], psq3[:, :, 64:128]
        cos_t = cos_sb[:, tb, :].rearrange("p (h z) -> p h z", h=HL)
        sin_t = sin_sb[:, tb, :].rearrange("p (h z) -> p h z", h=HL)
        qtm = P["qtm"].tile([128, HL, 128], bf, name="qtm")
        ta = P["tmp"].tile([128, HL, 64], f32, tag="rt", name="ta")
        tb_ = P["tmp"].tile([128, HL, 64], f32, tag="rt", name="tb_")
        nc.vector.tensor_mul(ta[:], qlo, cos_t)
        nc.vector.tensor_mul(tb_[:], qhi, sin_t)
        nc.vector.tensor_sub(qtm[:, :, 0:64], ta[:], tb_[:])
        tc_ = P["tmp"].tile([128, HL, 64], f32, tag="rt", name="tc_")
        td = P["tmp"].tile([128, HL, 64], f32, tag="rt", name="td")
        nc.vector.tensor_mul(tc_[:], qhi, cos_t)
        nc.vector.tensor_mul(td[:], qlo, sin_t)
        nc.vector.tensor_add(qtm[:, :, 64:128], tc_[:], td[:])

        klo, khi = pskv[:, 0:64], pskv[:, 64:128]
        cos_k = cos_sb[:, tb, 0:64]
        sin_k = sin_sb[:, tb, 0:64]
        ktm = P["ktm"].tile([128, 128], bf, name="ktm")
        ka = P["tmp"].tile([128, 64], f32, tag="rt", name="ka")
        kb = P["tmp"].tile([128, 64], f32, tag="rt", name="kb")
        nc.vector.tensor_mul(ka[:], klo, cos_k)
        nc.vector.tensor_mul(kb[:], khi, sin_k)
        nc.vector.tensor_sub(ktm[:, 0:64], ka[:], kb[:])
        kc = P["tmp"].tile([128, 64], f32, tag="rt", name="kc")
        kd = P["tmp"].tile([128, 64], f32, tag="rt", name="kd")
        nc.vector.tensor_mul(kc[:], khi, cos_k)
        nc.vector.tensor_mul(kd[:], klo, sin_k)
        nc.vector.tensor_add(ktm[:, 64:128], kc[:], kd[:])

        nc.any.tensor_copy(vh[:, tb, 0:128], pskv[:, 128:256])

        for h in range(HL):
            nc.scalar.dma_start_transpose(
                qT[:, h, tb * 128:(tb + 1) * 128], qtm[:, h, :])
        nc.scalar.dma_start_transpose(
            kT[:, tb * 128:(tb + 1) * 128], ktm[:])
    return qT, kT, vh


def _emit_attn(nc, P, qT, kT, vh):
    afm = P["afm"].tile([128, HL, S], bf, name="afm")
    for h in range(HL):
        for s in range(NS):
            pvs = [P["ps"].tile([128, 132], f32, tag="ps", name=f"pv{u}")
                   for u in range(4)]
            for j in range(4 * s + 4):
                pscore = P["ps"].tile([128, 512], f32, tag="ps", name="pscore")
                nc.tensor.matmul(
                    pscore[:],
                    kT[:, j * 128:(j + 1) * 128],
                    qT[:, h, s * 512:(s + 1) * 512],
                    start=True, stop=True)
                probs = P["probs"].tile([128, 512], bf, name="probs")
                nc.scalar.activation(
                    probs[:], pscore[:],
                    mybir.ActivationFunctionType.Exp, scale=SCALE)
                delta = j * 128 - s * 512
                if delta > -128:
                    # in-tile causal mask: keep col - row >= delta
                    nc.gpsimd.affine_select(
                        out=probs[:], in_=probs[:],
                        compare_op=mybir.AluOpType.is_ge,
                        fill=0.0, base=-delta,
                        pattern=[[1, 512]], channel_multiplier=-1)
                for u in range(4):
                    if 4 * s + u < j:
                        continue
                    nc.tensor.matmul(
                        pvs[u][:, 0:129],
                        probs[:, u * 128:(u + 1) * 128],
                        vh[:, j, 0:129],
                        start=(j == 0), stop=(j == 4 * s + u))
            for u in range(4):
                tq = s * 4 + u
                rcp = P["rcp"].tile([128, 1], f32, name="rcp")
                nc.vector.reciprocal(rcp[:], pvs[u][:, 128:129])
                atm = P["atm"].tile([128, 128], bf, name="atm")
                nc.vector.tensor_scalar_mul(atm[:], pvs[u][:, 0:128], rcp[:])
                nc.scalar.dma_start_transpose(
                    afm[:, h, tq * 128:(tq + 1) * 128], atm[:])
    return afm


def _emit_outproj(nc, P, ap, b, afm):
    for ob in range(E // 128):
        wot = P["wot"].tile([128, HL, 128], bf, name="wot")
        nc.sync.dma_start(
            wot[:],
            ap["wo"][:, ob * 128:(ob + 1) * 128].rearrange(
                "(h p) o -> p h o", p=128))
        for ts in range(NS):
            pso = P["ps"].tile([128, 512], f32, tag="ps", name="pso")
            for h in range(HL):
                nc.tensor.matmul(
                    pso[:],
                    wot[:, h, :],
                    afm[:, h, ts * 512:(ts + 1) * 512],
                    start=(h == 0), stop=(h == HL - 1))
            osb = P["osb"].tile([128, 512], f32, name="osb")
            nc.any.tensor_copy(osb[:], pso[:])
            nc.sync.dma_start(
                ap["po"][ob * 128:(ob + 1) * 128,
                         b * S + ts * 512: b * S + (ts + 1) * 512],
                osb[:])


def build_program(repeats=1):
    nc = bacc.Bacc("TRN2", target_bir_lowering=False, debug=False,
                   num_devices=N_CORES)

    ap = {
        "xT": nc.dram_tensor("xT", [E, B * S], bf, kind="ExternalInput").ap(),
        "wq": nc.dram_tensor("wq", [E, F], bf, kind="ExternalInput").ap(),
        "wo": nc.dram_tensor("wo", [HL * D, E], bf, kind="ExternalInput").ap(),
        "cosd": nc.dram_tensor("cosd", [S, HL * 64], bf,
                               kind="ExternalInput").ap(),
        "sind": nc.dram_tensor("sind", [S, HL * 64], bf,
                               kind="ExternalInput").ap(),
        "po": nc.dram_tensor("po", [E, B * S], f32, kind="ExternalOutput").ap(),
    }

    with tile.TileContext(nc) as tc:
        with (
            tc.tile_pool(name="const", bufs=1) as constp,
            tc.tile_pool(name="xcol", bufs=3) as xcolp,
            tc.tile_pool(name="qtm", bufs=3) as qtmp,
            tc.tile_pool(name="ktm", bufs=3) as ktmp,
            tc.tile_pool(name="tmp", bufs=6) as tmpp,
            tc.tile_pool(name="wot", bufs=3) as wotp,
            tc.tile_pool(name="qT", bufs=2) as qTp,
            tc.tile_pool(name="kT", bufs=2) as kTp,
            tc.tile_pool(name="vh", bufs=2) as vhp,
            tc.tile_pool(name="afm", bufs=2) as afmp,
            tc.tile_pool(name="probs", bufs=4) as probsp,
            tc.tile_pool(name="atm", bufs=4) as atmp,
            tc.tile_pool(name="rcp", bufs=4) as rcpp,
            tc.tile_pool(name="osb", bufs=4) as osbp,
            tc.tile_pool(name="ps", bufs=8, space="PSUM") as psp,
        ):
            P = {"const": constp, "xcol": xcolp, "qtm": qtmp, "ktm": ktmp,
                 "tmp": tmpp, "wot": wotp, "qT": qTp, "kT": kTp, "vh": vhp,
                 "afm": afmp, "probs": probsp, "atm": atmp, "rcp": rcpp,
                 "osb": osbp, "ps": psp}
            for _rep in range(repeats):
                _emit_iter(nc, P, ap)

    nc.compile()
    return nc


def prep_inputs(x, w_in, w_out):
    """Host-side shard + layout prep. Returns in_maps for the 8 cores."""
    x = np.asarray(x, dtype=np.float32)
    w_in = np.asarray(w_in, dtype=np.float32)
    w_out = np.asarray(w_out, dtype=np.float32)

    xT = np.ascontiguousarray(x.reshape(B * S, E).T).astype(BF16)

    # RoPE tables, fp32 math to match the reference
    inv_freq = (1.0 / (10000.0 ** (np.arange(0, D, 2, dtype=np.float32) / D))
                ).astype(np.float32)
    ang = np.arange(S, dtype=np.float32)[:, None] * inv_freq[None, :]
    cos1 = np.cos(ang).astype(np.float32)   # [S, 64]
    sin1 = np.sin(ang).astype(np.float32)
    cos4 = np.tile(cos1, (1, HL)).astype(BF16)   # [S, 256]
    sin4 = np.tile(sin1, (1, HL)).astype(BF16)

    in_maps = []
    for d in range(N_CORES):
        rows = np.concatenate([
            w_in[512 * d: 512 * d + 512],
            w_in[4096 + 128 * d: 4096 + 128 * d + 128],
            w_in[5120 + 128 * d: 5120 + 128 * d + 128],
        ], axis=0)                                    # [768, E]
        wq_d = np.ascontiguousarray(rows.T).astype(BF16)      # [E, 768]
        wo_d = np.ascontiguousarray(
            w_out[:, 512 * d: 512 * d + 512].T).astype(BF16)  # [512, E]
        in_maps.append({"xT": xT, "wq": wq_d, "wo": wo_d,
                        "cosd": cos4, "sind": sin4})
    return in_maps


_NC_CACHE = None


def get_program():
    global _NC_CACHE
    if _NC_CACHE is None:
        _NC_CACHE = build_program()
    return _NC_CACHE


def kernel(x, w_in, w_out):
    from concourse.bass_utils import run_bass_kernel_spmd
    nc = get_program()
    in_maps = prep_inputs(x, w_in, w_out)
    res = run_bass_kernel_spmd(nc, in_maps, list(range(N_CORES)))
    total = np.zeros((E, B * S), dtype=np.float32)
    for d in range(N_CORES):
        total += res.results[d]["po"]
    out = total.reshape(E, B, S).transpose(1, 2, 0)
    return np.ascontiguousarray(out, dtype=np.float32)
